# revision 3
# baseline (speedup 1.0000x reference)
"""DFMConv2d Trainium2 kernel.

Reference computation (per sample b):
  pooled = mean_{h,w} x[b]                          [C=256]
  h      = relu(pooled @ w1.T + b1)                 [128]
  mix    = softmax((h @ w2.T + b2).reshape(256, 8)) [256, 8]
  y      = conv3x3_SAME(x[b], base_filters)         [8, 64, 64]
  out[b] = einsum('on,nhw->ohw', mix, y)            [256, 64, 64]

Strategy (8 NeuronCores, data-parallel over batch, 8 samples/core), all
matmuls in float32r (~2e-4 rel err, full PE rate at N=512):

  conv:  y_tap[(t,n), hw] = sum_c filt[t,n,c] * x[c, hw] — taps in the
         stationary M dim (M=72), so x streams through the PE exactly
         twice (2 c-chunks); 16 matmuls/sample. PSUM chunks are copied
         into a zero-bordered 66x66 padded buffer (ACT, strided write).
  shift: z[(t,n), hw] = y_tap[(t,n), hw + shift(t)] via 9 per-tap
         SBUF->SBUF DMAs (free-dim window moves on the padded grid).
  mix:   out[o, hw] = mixT72.T @ z with K=72, where mixT72 replicates
         softmax(mix).T nine times via 4 doubling partition-shift DMAs.
  MLP/softmax run per sample in fp32; mixT built via PE transpose.
"""
import sys

sys.path.insert(0, "/opt/trn_rl_repo")

import numpy as np
import ml_dtypes

import concourse.bass as bass
import concourse.bacc as bacc
import concourse.tile as tile
import concourse.mybir as mybir
from concourse.bass_utils import run_bass_kernel_spmd
from contextlib import ExitStack

F32 = mybir.dt.float32
F32R = mybir.dt.float32r
BF16 = mybir.dt.bfloat16
AFT = mybir.ActivationFunctionType
AXX = mybir.AxisListType.X
ALU = mybir.AluOpType

N_CORES = 8
BPC = 8            # samples per core
C = 256
CO = 256
H = W = 64
HW = H * W
NB = 8             # n_base
HID = 128
CCH = 2            # channel chunks of 128
NHC = 8            # h-chunks (8 output rows each)
NT = 9             # taps
M72 = NT * NB      # 72
PW = 66            # padded width/height

_BUILT = None


def _build():
    nc = bacc.Bacc("TRN2", target_bir_lowering=False)

    d_x = nc.dram_tensor("x", [BPC, C, HW], F32R, kind="ExternalInput")
    d_w1t = nc.dram_tensor("w1t", [C, HID], F32, kind="ExternalInput")
    d_b1 = nc.dram_tensor("b1", [HID, 1], F32, kind="ExternalInput")
    d_w2p = nc.dram_tensor("w2p", [HID, NB, CO], F32, kind="ExternalInput")
    d_b2t = nc.dram_tensor("b2t", [128, 2, NB], F32, kind="ExternalInput")
    d_ft = nc.dram_tensor("ft", [128, CCH, M72], F32R, kind="ExternalInput")
    d_id = nc.dram_tensor("ident", [128, 128], F32, kind="ExternalInput")
    d_z0 = nc.dram_tensor("zeros", [128, PW], F32R, kind="ExternalInput")
    d_out = nc.dram_tensor("out", [BPC, 2, 128, HW], F32, kind="ExternalOutput")

    with tile.TileContext(nc) as tc, ExitStack() as ctx:
        prm = ctx.enter_context(tc.tile_pool(name="prm", bufs=1))
        xp = ctx.enter_context(tc.tile_pool(name="xp", bufs=2))
        ypp = ctx.enter_context(tc.tile_pool(name="ypp", bufs=2))
        zp = ctx.enter_context(tc.tile_pool(name="zp", bufs=2))
        op = ctx.enter_context(tc.tile_pool(name="op", bufs=3))
        sm = ctx.enter_context(tc.tile_pool(name="sm", bufs=2))
        ps_c = ctx.enter_context(tc.tile_pool(name="ps_c", bufs=2, space="PSUM"))
        ps_m = ctx.enter_context(tc.tile_pool(name="ps_m", bufs=3, space="PSUM"))
        ps_s = ctx.enter_context(tc.tile_pool(name="ps_s", bufs=2, space="PSUM"))

        # ---- params (loaded once) ----
        w1t_sb = prm.tile([128, CCH, HID], F32, tag="w1t")
        nc.sync.dma_start(out=w1t_sb, in_=d_w1t[:, :].rearrange("(cc p) h -> p cc h", p=128))
        b1_sb = prm.tile([128, 1], F32, tag="b1")
        nc.sync.dma_start(out=b1_sb, in_=d_b1[:, :])
        w2p_sb = prm.tile([HID, NB, CO], F32, tag="w2p")
        nc.sync.dma_start(out=w2p_sb, in_=d_w2p[:, :, :])
        b2t_sb = prm.tile([128, 2, NB], F32, tag="b2t")
        nc.sync.dma_start(out=b2t_sb, in_=d_b2t[:, :, :])
        ft_sb = prm.tile([128, CCH, M72], F32R, tag="ft")
        nc.sync.dma_start(out=ft_sb, in_=d_ft[:, :, :])
        id_sb = prm.tile([128, 128], F32, tag="ident")
        nc.sync.dma_start(out=id_sb, in_=d_id[:, :])
        z0_sb = prm.tile([128, PW], F32R, tag="z0")
        nc.sync.dma_start(out=z0_sb, in_=d_z0[:, :])
        pooled_sb = prm.tile([128, CCH, BPC], F32, tag="pooled")
        h_sb = prm.tile([128, BPC], F32, tag="h")

        for j in range(BPC):
            # ---- load + pooling ----
            xt = xp.tile([128, CCH, HW], F32R, tag="x")
            nc.sync.dma_start(
                out=xt, in_=d_x[j, :, :].rearrange("(cc p) hw -> p cc hw", p=128))
            for cc in range(CCH):
                nc.vector.reduce_sum(
                    pooled_sb[:, cc, j:j + 1], xt[:, cc, :].bitcast(F32), axis=AXX)

            # ---- attention MLP (fp32) ----
            ph = ps_s.tile([128, 1], F32, tag="sm")
            for cc in range(CCH):
                nc.tensor.matmul(ph, w1t_sb[:, cc, :], pooled_sb[:, cc, j:j + 1],
                                 start=(cc == 0), stop=(cc == 1))
            nc.scalar.activation(out=h_sb[:, j:j + 1], in_=ph, func=AFT.Relu,
                                 bias=b1_sb, scale=1.0)

            mixT_sb = sm.tile([M72, 2, 128], F32R, tag="mixT")
            for oc in range(2):
                pl = ps_s.tile([128, NB], F32, tag="sm")
                for n in range(NB):
                    nc.tensor.matmul(pl[:, n:n + 1],
                                     w2p_sb[:, n, oc * 128:(oc + 1) * 128],
                                     h_sb[:, j:j + 1], start=True, stop=True)
                lg_sb = sm.tile([128, NB], F32, tag="lg_sb")
                nc.vector.tensor_tensor(out=lg_sb, in0=pl, in1=b2t_sb[:, oc, :],
                                        op=ALU.add)
                ex_sb = sm.tile([128, NB], F32, tag="ex_sb")
                nc.scalar.activation(out=ex_sb, in_=lg_sb, func=AFT.Exp)
                sums = sm.tile([128, 1], F32, tag="sums")
                nc.vector.reduce_sum(sums, ex_sb, axis=AXX)
                rec = sm.tile([128, 1], F32, tag="rec")
                nc.vector.reciprocal(rec, sums)
                mix_sb = sm.tile([128, NB], F32, tag="mix_sb")
                nc.vector.tensor_scalar_mul(out=mix_sb, in0=ex_sb, scalar1=rec)
                ptr = ps_s.tile([NB, 128], F32, tag="sm")
                nc.tensor.transpose(ptr, mix_sb, id_sb)
                # rounded-to-f32r producer for the K=72 mix matmul
                nc.vector.tensor_copy(mixT_sb[0:NB, oc, :], ptr)
            # replicate rows [0:8) nine times via doubling partition-shift DMAs
            nc.sync.dma_start(out=mixT_sb[8:16], in_=mixT_sb[0:8])
            nc.sync.dma_start(out=mixT_sb[16:32], in_=mixT_sb[0:16])
            nc.sync.dma_start(out=mixT_sb[32:64], in_=mixT_sb[0:32])
            nc.sync.dma_start(out=mixT_sb[64:72], in_=mixT_sb[0:8])

            # ---- conv into padded y_tap (zero borders) ----
            ypad = ypp.tile([M72, PW * PW], F32R, tag="ypad")
            ypv = ypad.rearrange("p (h w) -> p h w", w=PW)
            nc.vector.tensor_copy(ypv[:, 0, :], z0_sb[0:M72, :])
            nc.vector.tensor_copy(ypv[:, PW - 1, :], z0_sb[0:M72, :])
            nc.vector.tensor_copy(ypv[:, :, 0:1].rearrange("p h w -> p (h w)"),
                                  z0_sb[0:M72, :])
            nc.vector.tensor_copy(ypv[:, :, PW - 1:PW].rearrange("p h w -> p (h w)"),
                                  z0_sb[0:M72, :])
            for hc in range(NHC):
                yps = ps_c.tile([128, 512], F32, tag="yps")
                for cc in range(CCH):
                    nc.tensor.matmul(yps[0:M72, :], ft_sb[:, cc, :],
                                     xt[:, cc, 512 * hc:512 * (hc + 1)],
                                     start=(cc == 0), stop=(cc == 1))
                nc.scalar.copy(
                    out=ypv[:, 1 + 8 * hc:1 + 8 * (hc + 1), 1:65].bitcast(F32),
                    in_=yps[0:M72, :].rearrange("p (h w) -> p h w", w=64))

            # ---- per-tap shifted windows into z ----
            zt = zp.tile([M72, HW], F32R, tag="z")
            for t, (dy, dx) in enumerate((dy, dx) for dy in range(3) for dx in range(3)):
                nc.sync.dma_start(
                    out=zt[NB * t:NB * (t + 1), :],
                    in_=ypv[NB * t:NB * (t + 1), dy:dy + 64, dx:dx + 64])

            # ---- mix: out[o, hw] = mixT72.T @ z (K=72, f32r) ----
            for oc in range(2):
                ot = op.tile([128, HW], F32, tag="out")
                for hc in range(NHC):
                    om = ps_m.tile([128, 512], F32, tag="ops")
                    nc.tensor.matmul(om, mixT_sb[:, oc, :],
                                     zt[:, 512 * hc:512 * (hc + 1)],
                                     start=True, stop=True)
                    eng = nc.vector if (hc % 2 == 0) else nc.scalar
                    if eng is nc.vector:
                        nc.vector.tensor_copy(ot[:, 512 * hc:512 * (hc + 1)], om)
                    else:
                        nc.scalar.copy(out=ot[:, 512 * hc:512 * (hc + 1)], in_=om)
                nc.sync.dma_start(out=d_out[j, oc, :, :], in_=ot)

    nc.compile()
    return nc


def _prep_inputs(x, w1, b1, w2, b2, base_filters):
    """Host-side input layout prep. Returns per-core in_maps."""
    B = x.shape[0]
    xs = np.ascontiguousarray(x.reshape(B, C, HW)).astype(np.float32)
    w1t = np.ascontiguousarray(w1.T).astype(np.float32) / float(HW)
    b1c = np.ascontiguousarray(b1.reshape(HID, 1)).astype(np.float32)
    w2p = np.ascontiguousarray(w2.reshape(CO, NB, HID).transpose(2, 1, 0)).astype(np.float32)
    b2t = np.ascontiguousarray(b2.reshape(2, 128, NB).transpose(1, 0, 2)).astype(np.float32)
    filt = base_filters.reshape(NB, C, 9)
    # ft[c_part, cc, 8*t + n] = filt[n, cc*128 + c_part, t]
    ft = np.ascontiguousarray(
        filt.reshape(NB, CCH, 128, NT).transpose(2, 1, 3, 0).reshape(128, CCH, M72)
    ).astype(np.float32)
    ident = np.eye(128, dtype=np.float32)
    zeros = np.zeros((128, PW), dtype=np.float32)

    in_maps = []
    for core in range(N_CORES):
        in_maps.append({
            "x": np.ascontiguousarray(xs[core * BPC:(core + 1) * BPC]),
            "w1t": w1t, "b1": b1c, "w2p": w2p, "b2t": b2t,
            "ft": ft, "ident": ident, "zeros": zeros,
        })
    return in_maps


def kernel(x, w1, b1, w2, b2, base_filters):
    global _BUILT
    if _BUILT is None:
        _BUILT = _build()
    nc = _BUILT
    in_maps = _prep_inputs(np.asarray(x, dtype=np.float32),
                           np.asarray(w1, dtype=np.float32),
                           np.asarray(b1, dtype=np.float32),
                           np.asarray(w2, dtype=np.float32),
                           np.asarray(b2, dtype=np.float32),
                           np.asarray(base_filters, dtype=np.float32))
    res = run_bass_kernel_spmd(nc, in_maps, core_ids=list(range(N_CORES)))
    outs = []
    for core in range(N_CORES):
        o = res.results[core]["out"]            # [BPC, 2, 128, HW]
        outs.append(o.reshape(BPC, CO, H, W))
    return np.concatenate(outs, axis=0).astype(np.float32)
